# revision 1
# baseline (speedup 1.0000x reference)
"""CapsuleLayer (dynamic routing) Trainium2 Bass kernel.

Sharding: pure data-parallel over batch B=256 -> 8 cores x 32 batches.
Per core the 32 batches run as 4 sub-chunks of 8; the SBUF partition dim
packs p = b*16 + ig where capsule index i = 16*g + ig, g in [0,72).

Phase 1 (u_hat = einsum('nidk,bik->bnid')): the K=8 contraction is packed
to K=128 by block-diagonalizing 16 capsules' inputs into the PE stationary
operand. The block-diagonal operand (15/16 zeros) is staged zero-padded on
the HOST and DMA'd straight to SBUF (DMA engines are otherwise idle), so no
on-chip masking/memset work is needed:
    lhsT[(ig,k), (b',ig')] = x[b', 16g+ig', k] * (ig==ig')
    rhs  = W2[(ig,k), (g,d,n)] = W[n, 16g+ig, d, k]
    psum[(b,ig), (d,n)] = u_hat[b, n, 16g+ig, d]
Matmuls write 6 g's into a 2-bank PSUM tile (3 g's per bank at 512-f32
stride) so the f32->bf16 copies amortize their fixed overhead; copies
alternate DVE / ACT. u_hat stays on-chip in bf16 as U[128, G, D, N].

Routing (3 iters, fused on-chip):
  outputs-einsum: DVE multiply (c bcast over d) + PE partition-reduce with
  per-sub-chunk 0/1 bmask32 stationaries whose columns land the 4 sub-chunks
  in disjoint rows of ONE [32, MMB, D, N] PSUM tile. Squash then runs once
  per round on 32 partitions (not 4x on 8), with the r0 1/N scale folded
  exactly into scalar_tensor_tensor ops. The outputs broadcast back to
  (b,ig) partitions via per-sub bcmask32 PE matmuls.
  agreement: DVE multiply (outputs bcast over g) + fold-tree d-reduction
  split DVE/GPSIMD. softmax over n: one ACT exp (bf16) + DVE reduce per
  round over the merged b-logit tile [128, NSUB, G, N].

Emission is step-major (lockstep) across the 4 sub-chunks: engines execute
their streams in order, so per-sub-chunk emission would serialize the whole
routing chain; lockstep keeps each engine's queue dense.
"""

import numpy as np
import ml_dtypes

B, N, I, D, DK = 256, 10, 1152, 16, 8
NCORES = 8
BC = B // NCORES      # 32 batches per core
BS = 8                # batches per sub-chunk
NSUB = BC // BS       # 4
IG = 16               # capsules per PE group
G = I // IG           # 72
ND = D * N            # 160, (d-major, n-minor)
GBLK = 12             # g per routing block
NBLK = G // GBLK      # 6
MMB = 3               # groups PSUM-accumulated per routing matmul (F=480)
GCP = 6               # g's per phase-1 PSUM tile (3 per bank x 2 banks)
NCP = G // GCP        # 12 copies per sub-chunk
import os
UCP_DVE = int(os.environ.get("K_UCP_DVE", "4"))   # every n-th U copy on DVE
POOL_FULL = {0: int(os.environ.get("K_PF0", "0")),
             1: int(os.environ.get("K_PF1", "0"))}
L2_POOL_ODD = os.environ.get("K_L2POOL", "0") == "1"
PH1_BUFS = int(os.environ.get("K_PH1B", "3"))
OBC_ACT = os.environ.get("K_OBCACT", "0") == "1"
RZB_ACT = os.environ.get("K_RZBACT", "0") == "1"
TM_POOL = {1: int(os.environ.get("K_TMPOOL1", "3")),
           2: int(os.environ.get("K_TMPOOL2", "3"))}
BF16 = ml_dtypes.bfloat16

_cache = {}


def _bcast(ap, axis, count):
    """Insert a stride-0 dim of size `count` at `axis`."""
    ap = ap.unsqueeze(axis)
    shape = list(ap.shape)
    shape[axis] = count
    return ap.broadcast_to(shape)


def _legalize_waits(nc):
    """This walrus build takes at most 1 embedded sync wait per TPB
    instruction (2 on EventSemaphore, 0 on Drain). Tile emits multi-wait
    sync_info; hoist the extras onto preceding EventSemaphore instructions
    on the same engine queue."""
    from concourse import mybir

    n = 0
    for fn in nc.m.functions:
        for blk in fn.blocks:
            out = []
            for inst in blk.instructions:
                si = inst.sync_info
                if si is not None and si.on_wait:
                    keep = 1
                    if inst.opcode == "Drain":
                        keep = 0
                    elif inst.opcode == "EventSemaphore":
                        keep = 2
                    w = list(si.on_wait)
                    if len(w) > keep:
                        extra = w[:len(w) - keep] if keep else w
                        kept = w[len(w) - keep:] if keep else []
                        for i0 in range(0, len(extra), 2):
                            n += 1
                            out.append(mybir.InstEventSemaphore(
                                name=f"{inst.name}-hw{n}",
                                engine=inst.engine, ins=[], outs=[],
                                sync_info=mybir.SyncInfo(
                                    on_wait=extra[i0:i0 + 2],
                                    on_update=[]),
                            ))
                        si.on_wait = kept
                out.append(inst)
            blk.instructions = out
    return n


def _build_nc():
    import concourse.bass as bass
    import concourse.tile as tile
    from concourse import mybir
    from contextlib import ExitStack

    f32 = mybir.dt.float32
    bf16 = mybir.dt.bfloat16
    AX = mybir.AxisListType
    OP = mybir.AluOpType
    AF = mybir.ActivationFunctionType

    def _view(ap, off, dims):
        """AP view at element offset `off` with explicit [stride, size]
        free dims (partition dim kept)."""
        return bass.AP(tensor=ap.tensor, offset=ap.offset + off,
                       ap=[list(ap.ap[0])] + [list(d) for d in dims])

    nc = bass.Bass()
    xb_d = nc.dram_tensor("xblk", [128, NSUB, G * BS * IG], bf16,
                          kind="ExternalInput")
    w2_d = nc.dram_tensor("w2", [128, G * ND], bf16, kind="ExternalInput")
    bm32_d = nc.dram_tensor("bmask32", [128, NSUB, NSUB * BS], bf16,
                            kind="ExternalInput")
    bc32_d = nc.dram_tensor("bcmask32", [32, NSUB, 128], bf16,
                            kind="ExternalInput")
    y_d = nc.dram_tensor("y", [NSUB * BS, ND], f32, kind="ExternalOutput")

    with tile.TileContext(nc) as tc:
        with ExitStack() as ctx:
            singles = ctx.enter_context(tc.tile_pool(name="singles", bufs=1))
            upool = ctx.enter_context(tc.tile_pool(name="upool", bufs=4))
            # PSUM for routing reserved up-front so the r0 output matmuls
            # need not wait for phase-1's PSUM banks to drain.
            outps = ctx.enter_context(
                tc.tile_pool(name="outps", bufs=1, space="PSUM"))
            bcps = ctx.enter_context(
                tc.tile_pool(name="bcps", bufs=(1 if PH1_BUFS > 2 else 2), space="PSUM"))

            GQ = G // 4
            # first xblk quarter + first w2 quarter lead the DMA queue so
            # the PE (and then the copy engines) start as early as possible
            actx = ExitStack()
            wpool = actx.enter_context(tc.tile_pool(name="wpool", bufs=1))
            xpool = actx.enter_context(tc.tile_pool(name="xpool", bufs=2))
            GD4 = G // 4
            xb0 = xpool.tile([128, G, BS, IG], bf16, tag="xblk")
            w2q = [wpool.tile([128, GQ * ND], bf16, tag=f"w2_{q}",
                              name=f"w2t_{q}")
                   for q in range(4)]
            # interleave the first sub-chunk's quarters with w2's so each
            # g-quarter's operands land just ahead of its matmuls
            for q in range(0, 4):
                nc.sync.dma_start(
                    _view(xb0, q * GD4 * BS * IG, [[1, GD4 * BS * IG]]),
                    xb_d[:, 0, q * GD4 * BS * IG:(q + 1) * GD4 * BS * IG])
                nc.sync.dma_start(
                    w2q[q], w2_d[:, q * GQ * ND:(q + 1) * GQ * ND])
            bm32 = singles.tile([128, NSUB, NSUB * BS], bf16)
            nc.sync.dma_start(bm32, bm32_d[:])
            bc32 = singles.tile([32, NSUB, 128], bf16)
            nc.sync.dma_start(bc32, bc32_d[:])

            Us = []
            po0 = outps.tile([32, MMB, D, N], f32, tag="po", name="po0")
            po_k = [0]

            def po_chain(po, s, rhs_tile, nj, last_k):
                """Accumulate `nj` MMB-wide column groups of sub-chunk s
                into the shared [32, ...] po tile."""
                for j in range(nj):
                    nc.tensor.matmul(
                        po, bm32[:, s], rhs_tile[:, j * MMB:(j + 1) * MMB],
                        start=(po_k[0] == 0), stop=(po_k[0] == last_k),
                        skip_group_check=True)
                    po_k[0] += 1

            # ---------------- Phase A: u_hat build ----------------
            if True:
                ph1ps = actx.enter_context(
                    tc.tile_pool(name="ph1ps", bufs=PH1_BUFS, space="PSUM"))

                r0_last = NSUB * (G // MMB) - 1
                for s in range(NSUB):
                    if s == 0:
                        xb = xb0
                    else:
                        xb = xpool.tile([128, G, BS, IG], bf16, tag="xblk")
                        for q in range(4):
                            nc.sync.dma_start(
                                _view(xb, q * GD4 * BS * IG,
                                      [[1, GD4 * BS * IG]]),
                                xb_d[:, s, q * GD4 * BS * IG:
                                     (q + 1) * GD4 * BS * IG])
                    U = upool.tile([128, G, D, N], bf16, tag="U")
                    Us.append(U)
                    for cpi in range(NCP):
                        ps = ph1ps.tile([128, 1024], f32, tag="ph1")
                        for j in range(GCP):
                            g = cpi * GCP + j
                            q, gq = g // GQ, g % GQ
                            nc.tensor.matmul(
                                _view(ps, (j // 3) * 512 + (j % 3) * ND,
                                      [[1, ND]]),
                                xb[:, g],
                                w2q[q][:, gq * ND:(gq + 1) * ND],
                                start=True, stop=True)
                        src = _view(ps, 0, [[512, 2], [ND, 3], [1, ND]])
                        dst = _view(U, cpi * GCP * ND,
                                    [[MMB * ND, 2], [ND, 3], [1, ND]])
                        if cpi % UCP_DVE == 0:
                            nc.vector.tensor_copy(dst, src)
                        else:
                            nc.scalar.copy(dst, src)
                    # r0 outputs-einsum for the previous sub-chunk (its U
                    # copies drained while this sub-chunk's matmuls ran)
                    if s > 0:
                        po_chain(po0, s - 1, Us[s - 1], G // MMB, r0_last)
                po_chain(po0, NSUB - 1, Us[NSUB - 1], G // MMB, r0_last)
            actx.close()

            # ---------------- Routing pools ----------------
            tpool = ctx.enter_context(tc.tile_pool(name="tpool", bufs=1))
            tfpool = ctx.enter_context(tc.tile_pool(name="tfpool", bufs=1))
            blpool = ctx.enter_context(tc.tile_pool(name="blpool", bufs=1))
            smpool = ctx.enter_context(tc.tile_pool(name="smpool", bufs=1))
            cpool = ctx.enter_context(tc.tile_pool(name="cpool", bufs=1))
            obcpool = ctx.enter_context(tc.tile_pool(name="obc", bufs=4))
            tiny = ctx.enter_context(tc.tile_pool(name="tiny", bufs=1))

            bl = blpool.tile([128, NSUB, G, N], f32, tag="bl")

            def squash_all(po, r):
                """Squash the merged [32, MMB, D, N] po tile; returns the
                per-sub obc broadcasts (or None after the final round).
                The r0 uniform-softmax 1/N scale is folded exactly into
                the scalar_tensor_tensor ops (squash is not scale-inv)."""
                alpha = 1.0 / N if r == 0 else 1.0
                v = tiny.tile([32, D, N], f32, tag="v", name=f"v{r}")
                nc.vector.tensor_reduce(
                    _view(v, 0, [[1, ND]]),
                    _view(po, 0, [[1, ND], [ND, MMB]]),
                    axis=AX.X, op=OP.add)
                vsq = tiny.tile([32, D, N], f32, tag="vsq", name=f"vsq{r}")
                nc.vector.scalar_tensor_tensor(
                    vsq, v, alpha * alpha, v, op0=OP.mult, op1=OP.mult)
                nsq = tiny.tile([32, N], f32, tag="ns", name=f"ns{r}")
                nc.vector.tensor_reduce(
                    nsq, vsq.transpose([0, 2, 1]), axis=AX.X, op=OP.add)
                sq = tiny.tile([32, N], f32, tag="sq", name=f"sq{r}")
                nc.scalar.sqrt(sq, nsq)
                t1 = tiny.tile([32, N], f32, tag="t1", name=f"t1{r}")
                nc.vector.scalar_tensor_tensor(
                    t1, nsq, 1.0, sq, op0=OP.add, op1=OP.mult)
                rec = tiny.tile([32, N], f32, tag="rec", name=f"rec{r}")
                nc.vector.reciprocal(rec, t1)
                fac = tiny.tile([32, N], f32, tag="fac", name=f"fac{r}")
                nc.vector.tensor_mul(fac, nsq, rec)
                if r == 2:
                    ov = tiny.tile([32, D, N], f32, tag="ovf", name="ovf")
                    nc.vector.scalar_tensor_tensor(
                        ov, v, alpha, _bcast(fac, 1, D),
                        op0=OP.mult, op1=OP.mult)
                    nc.sync.dma_start(y_d[:], _view(ov, 0, [[1, ND]]))
                    return None
                ov = tiny.tile([32, D, N], bf16, tag="ov", name=f"ov{r}")
                nc.vector.scalar_tensor_tensor(
                    ov, v, alpha, _bcast(fac, 1, D), op0=OP.mult, op1=OP.mult)
                obc = {}
                psb = {}
                for s in range(NSUB):
                    psb[s] = bcps.tile([128, D, N], f32, tag="bc",
                                       name=f"bc{r}{s}")
                    nc.tensor.matmul(psb[s], bc32[:, s], ov,
                                     start=True, stop=True)
                for s in range(NSUB):
                    obc[s] = obcpool.tile([128, D, N], bf16, tag="obc",
                                          name=f"obc{r}{s}")
                    if OBC_ACT or s % 2 == 1:
                        nc.scalar.copy(obc[s], psb[s])
                    else:
                        nc.vector.tensor_copy(obc[s], psb[s])
                return obc

            AGRW = 10  # (s, blk) pairs in flight per agreement window

            def _pool_full_pairs(n):
                """Agreement pairs Pool owns end-to-end. Taken from the LAST
                sub-chunks so Pool's chunky muls land at the round tail,
                overlapping the next round's softmax/tm work on DVE/ACT
                instead of blocking the L3/L4 folds everyone needs."""
                ordered = [(s, blk) for s in (3, 2, 1, 0)
                           for blk in (5, 3, 1, 4, 2, 0)]
                return set(ordered[:n])

            def agreement_all(obcs, first, pool_n=0):
                """bl[:, s] (+)= sum_d U[s] * obc[s]. s-major so sub-chunk
                0's logits complete first and the next round's softmax can
                start while later sub-chunks still fold. A few pairs run
                end-to-end on Pool to offload the DVE; the rest split the
                fold tree DVE (L1/L2) -> Pool (L3/L4)."""
                pool_own = _pool_full_pairs(pool_n)
                pairs = [(s, blk) for s in range(NSUB)
                         for blk in range(NBLK)]
                for w0 in range(0, len(pairs), AGRW):
                    win = pairs[w0:w0 + AGRW]
                    t2s, t2fs = {}, {}
                    for s, blk in win:
                        g0 = blk * GBLK
                        ri = (s * NBLK + blk) % 12
                        eng = nc.gpsimd if (s, blk) in pool_own else \
                            nc.vector
                        t2 = tpool.tile([128, GBLK, D, N], bf16,
                                        tag=f"t2_{ri}",
                                        name=f"t2_{w0}_{blk}_{s}")
                        eng.tensor_mul(
                            t2, Us[s][:, g0:g0 + GBLK],
                            _bcast(obcs[s], 1, GBLK))
                        t2s[(s, blk)] = t2
                    for s, blk in win:
                        t2 = t2s[(s, blk)]
                        ri = (s * NBLK + blk) % 12
                        eng = nc.gpsimd if (s, blk) in pool_own else \
                            nc.vector
                        t2f = tfpool.tile([128, GBLK, 8, N], bf16,
                                          tag=f"t2f_{ri}",
                                          name=f"t2f_{w0}_{blk}_{s}")
                        eng.tensor_add(
                            t2f, t2[:, :, 0:8], t2[:, :, 8:16])
                        t2fs[(s, blk)] = t2f
                    for s, blk in win:
                        t2f = t2fs[(s, blk)]
                        eng = nc.gpsimd if (
                            (L2_POOL_ODD and s % 2 == 1) or
                            (s, blk) in pool_own) else nc.vector
                        eng.tensor_add(
                            t2f[:, :, 0:4], t2f[:, :, 0:4], t2f[:, :, 4:8])
                    for s, blk in win:
                        t2f = t2fs[(s, blk)]
                        nc.gpsimd.tensor_add(
                            t2f[:, :, 0:2], t2f[:, :, 0:2], t2f[:, :, 2:4])
                    for s, blk in win:
                        t2f = t2fs[(s, blk)]
                        g0 = blk * GBLK
                        if first:
                            nc.gpsimd.tensor_add(
                                bl[:, s, g0:g0 + GBLK],
                                t2f[:, :, 0], t2f[:, :, 1])
                        else:
                            nc.gpsimd.tensor_add(
                                t2f[:, :, 0], t2f[:, :, 0], t2f[:, :, 1])
                            nc.gpsimd.tensor_add(
                                bl[:, s, g0:g0 + GBLK],
                                bl[:, s, g0:g0 + GBLK], t2f[:, :, 0])

            # ---- r=0: c uniform -> outputs = squash(mean_i u_hat) ----
            obcs = squash_all(po0, 0)
            agreement_all(obcs, first=True, pool_n=POOL_FULL[0])

            # ---- r = 1, 2 ----
            for r in (1, 2):
                # per-s softmax: sub-chunk s's chain starts as soon as its
                # logits finish folding, overlapping later sub-chunks
                es = smpool.tile([128, NSUB, G, N], bf16, tag="e",
                                 name=f"e{r}")
                zs = smpool.tile([128, NSUB, G], f32, tag="z", name=f"z{r}")
                rz = smpool.tile([128, NSUB, G], f32, tag="rz",
                                 name=f"rz{r}")
                rzb = smpool.tile([128, NSUB, G], bf16, tag="rzb",
                                  name=f"rzb{r}")
                cs = cpool.tile([128, NSUB, G, N], bf16, tag="c",
                                name=f"c{r}")
                for s in range(NSUB):
                    nc.scalar.activation(es[:, s], bl[:, s], AF.Exp)
                    nc.vector.tensor_reduce(
                        zs[:, s], es[:, s], axis=AX.X, op=OP.add)
                    nc.vector.reciprocal(rz[:, s], zs[:, s])
                    if RZB_ACT:
                        nc.scalar.copy(rzb[:, s], rz[:, s])
                    else:
                        nc.vector.tensor_copy(rzb[:, s], rz[:, s])
                    nc.vector.tensor_mul(
                        cs[:, s], es[:, s], _bcast(rzb[:, s], 2, N))

                po = outps.tile([32, MMB, D, N], f32, tag="po",
                                name=f"po{r}")
                po_k[0] = 0
                last_k = NBLK * NSUB * (GBLK // MMB) - 1
                # late-sub tm muls offloaded to the (otherwise idle) Pool:
                # they start as soon as cs[s] lands and finish before the
                # po chain reaches them
                tm_pool = {(3 - i // 3, 5 - i % 3) for i in range(TM_POOL[r])}
                for s in range(NSUB):
                    for blk in range(NBLK):
                        g0 = blk * GBLK
                        tm = tpool.tile([128, GBLK, D, N], bf16,
                                        tag=f"t2_{(s * NBLK + blk) % 12}",
                                        name=f"tm{r}_{s}_{blk}")
                        eng = nc.gpsimd if (s, blk) in tm_pool else nc.vector
                        eng.tensor_mul(
                            tm, Us[s][:, g0:g0 + GBLK],
                            _bcast(cs[:, s, g0:g0 + GBLK], 2, D))
                        po_chain(po, s, tm, GBLK // MMB, last_k)
                obcs = squash_all(po, r)
                if r == 1:
                    agreement_all(obcs, first=False, pool_n=POOL_FULL[1])
    _legalize_waits(nc)
    return nc


def _prep_inputs(inputs, W):
    """Host-side layout prep. Returns per-core input maps."""
    W = np.asarray(W, dtype=np.float32)
    inputs = np.asarray(inputs, dtype=np.float32)
    # W2[(ig,k), (g,d,n)] = W[n, 16g+ig, d, k]
    Wr = W.reshape(N, G, IG, D, DK)
    w2 = np.ascontiguousarray(
        Wr.transpose(2, 4, 1, 3, 0)).reshape(128, G * ND).astype(BF16)
    # bmask32[(b,ig), s, 8s'+b'] = (b==b')(s==s'): lands sub-chunk s in
    # rows [8s, 8s+8) of the merged po tile
    bm32 = np.zeros((BS, IG, NSUB, NSUB, BS), np.float32)
    for s in range(NSUB):
        for b in range(BS):
            bm32[b, :, s, s, b] = 1.0
    bm32 = bm32.reshape(128, NSUB, NSUB * BS).astype(BF16)
    # bcmask32[8s'+b', s, (b,ig)] = (b==b')(s==s'): broadcasts row 8s+b of
    # ov back to the (b, ig) partitions
    bc32 = np.zeros((NSUB, BS, NSUB, BS, IG), np.float32)
    for s in range(NSUB):
        for b in range(BS):
            bc32[s, b, s, b, :] = 1.0
    bc32 = bc32.reshape(32, NSUB, 128).astype(BF16)

    in_maps = []
    for cc in range(NCORES):
        xcore = inputs[cc * BC:(cc + 1) * BC]       # [32, 1152, 8]
        xr = xcore.reshape(NSUB, BS, G, IG, DK)     # [s, b, g, ig, k]
        # zero-padded block-diagonal stationary, staged on host:
        # xq[(ig,k), s, (g, b, ig')] = x[s*8+b, 16g+ig, k] * (ig==ig')
        xq = np.zeros((IG, DK, NSUB, G, BS, IG), np.float32)
        for ig in range(IG):
            xq[ig, :, :, :, :, ig] = xr[:, :, :, ig, :].transpose(3, 0, 2, 1)
        xq = xq.reshape(128, NSUB, G * BS * IG).astype(BF16)
        in_maps.append(
            {"xblk": xq, "w2": w2, "bmask32": bm32, "bcmask32": bc32})
    return in_maps


def _run(inputs, W, trace=False):
    from concourse.bass_utils import run_bass_kernel_spmd

    if "nc" not in _cache:
        _cache["nc"] = _build_nc()
    nc = _cache["nc"]
    in_maps = _prep_inputs(inputs, W)
    res = run_bass_kernel_spmd(
        nc, in_maps, core_ids=list(range(NCORES)), trace=trace)
    # y[(s, b), (d, n)] per core -> out[b_global, n, d]
    out = np.empty((B, N, D), np.float32)
    for cc in range(NCORES):
        yc = res.results[cc]["y"].reshape(BC, D, N)
        out[cc * BC:(cc + 1) * BC] = yc.transpose(0, 2, 1)
    return out, res


def kernel(inputs, W):
    out, _ = _run(inputs, W, trace=False)
    return out

